# revision 1
# baseline (speedup 1.0000x reference)
"""Trainium2 Bass kernel for nn_CrossAttention (B=16, LQ=64, LD=2048, D=256).

Math (per batch), restructured from the reference to avoid the [LD, LD]
intermediate:
    S^T = (q*w3 + w1-fold).T-matmul(doc^T) + sq bias      [LQ, LD]
    E = exp(S^T)             (no max-subtraction; |S| <= ~8)
    sum1[j] = sum_i E[j, i]  (free-axis accum of the exp)
    E_nat = E^T (PE transpose), sum2[i] = sum_j E_nat[i, j]
    S_Q2D = E_nat / sum2
    T = S_Q2D^T @ doc        [LQ, D]
    A_D2Q = E_nat @ (query/sum1);  A_Q2D = E_nat @ (T/sum1)
    V = [doc, A_D2Q, doc*A_D2Q, doc*A_Q2D]

Sharding: batch dim across 8 cores (2 batches per core, stacked on the
64-partition halves wherever profitable). Matmuls in float32r (full PE
rate, ~1.6e-4 rel err); everything else fp32.
"""
import numpy as np

import concourse.bass as bass
import concourse.bacc as bacc
import concourse.mybir as mybir
from concourse.tile import TileContext
from concourse.bass_utils import run_bass_kernel_spmd

B, LQ, LD, D = 16, 64, 2048, 256
NCORES, BPC = 8, B // 8
NCH = LD // 128  # 16 chunks of 128 doc rows per batch

F32 = mybir.dt.float32
F32R = mybir.dt.float32r
EXP = mybir.ActivationFunctionType.Exp
MULT = mybir.AluOpType.mult


def _bc(ap, n):
    """Broadcast an AP along a new trailing free dim of size n (step 0)."""
    return bass.AP(ap.tensor, ap.offset, list(ap.ap) + [[0, n]])


def _body(nc, tc, query_d, doc_d, docT_d, w_d, v_d, ident_d):
    sb = tc.alloc_tile_pool(name="sb", bufs=1)
    vpool = tc.alloc_tile_pool(name="vp", bufs=3)
    ps = tc.alloc_tile_pool(name="ps", bufs=6, space="PSUM")
    stp = tc.alloc_tile_pool(name="stp", bufs=2, space="PSUM")

    # ---- static SBUF tensors -------------------------------------------------
    ident = sb.tile([128, 128], F32, tag="ident")
    nc.sync.dma_start(ident[:], ident_d.ap())

    w_sb = sb.tile([128, 6], F32, tag="w")  # cols: w1(h0,h1) w2(h0,h1) w3(h0,h1)
    nc.sync.dma_start(w_sb[:].rearrange("p (k o) -> p k o", o=1),
                      w_d.ap().rearrange("(k p) o -> p k o", p=128))

    q_sb = sb.tile([128, D], F32R, tag="q")  # batches stacked on partitions
    nc.sync.dma_start(q_sb[0:64, :], query_d.ap()[0])
    nc.sync.dma_start(q_sb[64:128, :], query_d.ap()[1])

    docT = [[sb.tile([128, LD], F32R, tag=f"docT{b}{h}", name=f"docT{b}{h}")
             for h in range(2)] for b in range(BPC)]  # [p=d%128, i] per (batch, d-half)
    for iq in range(4):
        for b in range(BPC):
            for h in range(2):
                nc.sync.dma_start(
                    docT[b][h][:, iq * 512:(iq + 1) * 512],
                    docT_d.ap()[b][h][:, iq * 512:(iq + 1) * 512])

    doc_sb = []
    for b in range(BPC):
        t = sb.tile([128, NCH * D], F32R, tag=f"doc{b}", name=f"doc{b}")  # [p,(c,d)]
        nc.sync.dma_start(t[:].rearrange("p (c d) -> p c d", d=D),
                          doc_d.ap()[b].rearrange("(c p) d -> p c d", p=128))
        doc_sb.append(t)

    E1 = sb.tile([128, LD], F32R, tag="E1")          # [j2, i] stacked pair
    SQ2D = sb.tile([128, NCH * 128], F32R, tag="SQ2D")  # [i%128, (c, j2)]
    qw3T = sb.tile([128, 4 * 128], F32R, tag="qw3T")  # block-diag M1 lhsT
    qT_sb = sb.tile([128, 4 * 64], F32, tag="qT")    # [d%128, (b,h,j)]
    qn_sb = sb.tile([128, D], F32R, tag="qn")   # query/sum1, stacked
    tn_sb = sb.tile([128, D], F32R, tag="tn")   # T/sum1, stacked
    sq_sb = sb.tile([128, 1], F32, tag="sq")
    s1p = sb.tile([128, 4], F32, tag="s1p")
    sum1 = sb.tile([128, 1], F32, tag="sum1")
    r1 = sb.tile([128, 1], F32, tag="r1")
    s2 = sb.tile([128, 2 * NCH], F32, tag="s2")
    r2 = sb.tile([128, 2 * NCH], F32, tag="r2")

    # ---- query prep: qT, qw3T (w3*qT + w1, block-diag), sq -------------------
    qt_ps = ps.tile([128, 512], F32, tag="ps")
    for b in range(BPC):
        for h in range(2):
            k = h * 2 + b
            nc.tensor.transpose(
                qt_ps[:, k * 64:(k + 1) * 64],
                q_sb[b * 64:(b + 1) * 64, h * 128:(h + 1) * 128].bitcast(F32),
                ident[b * 64:(b + 1) * 64, b * 64:(b + 1) * 64],
            )
    nc.scalar.copy(qT_sb[:], qt_ps[:, 0:256])
    for b in range(BPC):
        for h in range(2):
            k = b * 2 + h          # K-chunk order stays (b, h)
            t = h * 2 + b          # qt_ps block order is (h, b)
            # filled block: qT*w3 + w1 ; complementary block: zeros (qT*0)
            nc.vector.tensor_scalar(
                qw3T[:, k * 128 + b * 64: k * 128 + b * 64 + 64],
                qt_ps[:, t * 64:(t + 1) * 64],
                w_sb[:, 4 + h:5 + h], w_sb[:, 0 + h:1 + h],
                MULT, mybir.AluOpType.add,
            )
            nc.vector.tensor_scalar(
                qw3T[:, k * 128 + (1 - b) * 64: k * 128 + (1 - b) * 64 + 64],
                qt_ps[:, t * 64:(t + 1) * 64],
                0.0, None, MULT,
            )
    # sq[j2] = query @ w2 : lhsT = qT arranged [d-half, (b, j)]
    sq_ps = ps.tile([128, 512], F32, tag="ps")
    for h in range(2):
        nc.tensor.matmul(sq_ps[:, 0:1], qT_sb[:, h * 128:(h + 1) * 128],
                         w_sb[:, 2 + h:3 + h], start=(h == 0), stop=(h == 1))
    nc.scalar.copy(sq_sb[:], sq_ps[:, 0:1])

    # ---- M1 (S^T) + exp, in i-quarters ---------------------------------------
    for iq in range(4):
        st = stp.tile([128, 512], F32, tag="st")
        for k in range(4):
            b, h = k >> 1, k & 1
            nc.tensor.matmul(
                st[:],
                qw3T[:, k * 128:(k + 1) * 128],
                docT[b][h][:, iq * 512:(iq + 1) * 512],
                start=(k == 0), stop=(k == 3),
            )
        nc.scalar.activation(E1[:, iq * 512:(iq + 1) * 512], st[:], EXP,
                             bias=sq_sb[:, 0:1], accum_out=s1p[:, iq:iq + 1])

    nc.vector.tensor_add(sum1[:], s1p[:, 0:1], s1p[:, 1:2])
    nc.vector.tensor_add(sum1[:], sum1[:], s1p[:, 2:3])
    nc.vector.tensor_add(sum1[:], sum1[:], s1p[:, 3:4])
    nc.vector.reciprocal(r1[:], sum1[:])

    # qn is ready as soon as r1 is -> lets M2 (A_D2Q) run during the M3 phase
    nc.vector.tensor_scalar(qn_sb[:], q_sb[:].bitcast(F32), r1[:, 0:1], None, MULT)

    # ---- E_nat chunks: transpose, sum2, normalize -> SQ2D; M3 (T); early M2 --
    # T via full-width lhsT: tA rows 0:64 = T_b0 (rows 64:128 cross-batch junk),
    # tB rows 64:128 = T_b1. Keeps every consumer partition-aligned.
    tA = ps.tile([128, 512], F32, tag="ps", name="tA")
    tB = ps.tile([128, 512], F32, tag="ps", name="tB")
    vv = [v_d.ap()[b].rearrange("(c p) e -> p c e", p=128) for b in range(BPC)]
    ve = {}
    for c in range(NCH):
        en = ps.tile([128, 512], F32, tag="ps")
        nc.tensor.transpose(en[:, 0:128], E1[:, c * 128:(c + 1) * 128].bitcast(F32),
                            ident[:])
        en3 = en[:, 0:128].rearrange("p (k j) -> p k j", k=2)
        nc.vector.tensor_reduce(s2[:, c * 2:(c + 1) * 2], en3,
                                mybir.AxisListType.X, mybir.AluOpType.add)
        nc.vector.reciprocal(r2[:, c * 2:(c + 1) * 2], s2[:, c * 2:(c + 1) * 2])
        nc.vector.tensor_tensor(
            SQ2D[:, c * 128:(c + 1) * 128].rearrange("p (k j) -> p k j", k=2),
            en3, _bc(r2[:, c * 2:(c + 1) * 2], 64), MULT)
        nc.tensor.matmul(tA[:, 0:256], SQ2D[:, c * 128:(c + 1) * 128],
                         doc_sb[0][:, c * D:(c + 1) * D],
                         start=(c == 0), stop=(c == NCH - 1))
        nc.tensor.matmul(tB[:, 0:256], SQ2D[:, c * 128:(c + 1) * 128],
                         doc_sb[1][:, c * D:(c + 1) * D],
                         start=(c == 0), stop=(c == NCH - 1))
        # early M2: A_D2Q chunk + V slots 2-3, drained per chunk-pair
        u = c % 2
        for b in range(BPC):
            if u == 0:
                ve[b] = vpool.tile([128, 2 * 2 * D], F32, tag="ve", name=f"ve{b}_{c}")
            a2 = ps.tile([128, 512], F32, tag="ps", name=f"a2_{b}{c}")
            nc.tensor.matmul(a2[:, 0:D],
                             E1[b * 64:(b + 1) * 64, c * 128:(c + 1) * 128],
                             qn_sb[b * 64:(b + 1) * 64, :], start=True, stop=True)
            o = u * 2 * D
            nc.scalar.copy(ve[b][:, o:o + D], a2[:, 0:D])
            doc_c = doc_sb[b][:, c * D:(c + 1) * D].bitcast(F32)
            nc.vector.tensor_mul(ve[b][:, o + D:o + 2 * D], doc_c, ve[b][:, o:o + D])
            if u == 1:
                nc.sync.dma_start(
                    vv[b][:, c - 1:c + 1, D:3 * D],
                    ve[b][:].rearrange("p (t e) -> p t e", e=2 * D))

    # slot 1 (doc) goes out as one big strided DMA per batch
    for b in range(BPC):
        nc.sync.dma_start(vv[b][:, :, 0:D],
                          doc_sb[b][:].rearrange("p (c d) -> p c d", d=D).bitcast(F32))

    # ---- Tn, then M4 (A_Q2D) + V slot 4 --------------------------------------
    nc.vector.tensor_scalar(tn_sb[0:64, :], tA[0:64, 0:256], r1[0:64, 0:1],
                            None, MULT)
    nc.vector.tensor_scalar(tn_sb[64:128, :], tB[64:128, 0:256],
                            r1[64:128, 0:1], None, MULT)
    for cg in range(NCH // 4):
        for b in range(BPC):
            vl = vpool.tile([128, 4 * D], F32, tag="vl", name=f"vl{b}_{cg}")
            for t in range(4):
                c = cg * 4 + t
                a4 = ps.tile([128, 512], F32, tag="ps", name=f"a4_{b}{c}")
                nc.tensor.matmul(a4[:, 0:D],
                                 E1[b * 64:(b + 1) * 64, c * 128:(c + 1) * 128],
                                 tn_sb[b * 64:(b + 1) * 64, :], start=True, stop=True)
                doc_c = doc_sb[b][:, c * D:(c + 1) * D].bitcast(F32)
                nc.vector.tensor_mul(vl[:, t * D:(t + 1) * D], doc_c, a4[:, 0:D])
            nc.sync.dma_start(
                vv[b][:, cg * 4:(cg + 1) * 4, 3 * D:4 * D],
                vl[:].rearrange("p (t e) -> p t e", e=D))

    for p in (stp, ps, vpool, sb):
        p.release()


def _build():
    nc = bacc.Bacc("TRN2", target_bir_lowering=False, debug=False,
                   num_devices=NCORES)
    query_d = nc.declare_dram_parameter("query", [BPC, LQ, D], F32R, isOutput=False)
    doc_d = nc.declare_dram_parameter("doc", [BPC, LD, D], F32R, isOutput=False)
    docT_d = nc.declare_dram_parameter("docT", [BPC, 2, 128, LD], F32R, isOutput=False)
    w_d = nc.declare_dram_parameter("W", [3 * D, 1], F32, isOutput=False)
    v_d = nc.declare_dram_parameter("V", [BPC, LD, 4 * D], F32, isOutput=True)
    ident_d = nc.inline_tensor(np.eye(128, dtype=np.float32), name="ident128")
    with TileContext(nc) as tc:
        _body(nc, tc, query_d, doc_d, docT_d, w_d, v_d, ident_d)
    nc.finalize()
    return nc


_CACHE = {}


def run(query, doc, W, trace=False):
    if "nc" not in _CACHE:
        _CACHE["nc"] = _build()
    nc = _CACHE["nc"]
    docT = np.ascontiguousarray(doc.transpose(0, 2, 1)).reshape(B, 2, 128, LD)
    in_maps = [
        {
            "query": np.ascontiguousarray(query[c * BPC:(c + 1) * BPC]),
            "doc": np.ascontiguousarray(doc[c * BPC:(c + 1) * BPC]),
            "docT": docT[c * BPC:(c + 1) * BPC],
            "W": np.ascontiguousarray(W),
        }
        for c in range(NCORES)
    ]
    res = run_bass_kernel_spmd(nc, in_maps, list(range(NCORES)), trace=trace)
    out = np.concatenate([res.results[c]["V"] for c in range(NCORES)], axis=0)
    return out.astype(np.float32, copy=False), res


def kernel(query, doc, W):
    out, _ = run(query, doc, W)
    return out

